# revision 15
# baseline (speedup 1.0000x reference)
# GAT layer kernel for Trainium2 (Bass/Tile), data-parallel over batch:
# one graph per NeuronCore, 8 cores.
#
# Math (per graph, N=2048 nodes, F=128 in, O=64 out):
#   Wh = h @ W + b
#   e[i,j] = leakyrelu(s1[i] + s2[j], 0.2),  s1 = Wh@a1, s2 = Wh@a2
#   att = softmax over i of where(adj>0, e, -inf)
#   out = elu(att^T @ Wh)
#
# Kernel formulation:
#   s1 = h@(W a1), s2 = h@(W a2) come for free as extra columns of the
#   stage-1 matmul [W | 0 | w1 | w2].  The softmax over i is invariant to
#   per-column-j scaling, so divide exp(leakyrelu(t)) = max(e^t, e^0.2t)
#   by f2[j] = exp(0.2(s2+b.a2)):
#     P[i,j] = adj[i,j] * max(e1[i]*q2[j], f1[i]),
#     e1 = exp(s1+b.a1), f1 = exp(0.2(s1+b.a1)), q2 = exp(0.8(s2+b.a2)).
#   This is EXACT (no approximation) and needs exp only on N-vectors.
#   P is materialized per 128-row block: on DVE either a fused custom op
#   (fp32 path) or a bf16 tensor_scalar (4x mode) + tensor_tensor (2x)
#   pair, with a trailing column slice offloaded to the Pool engine.
#   TensorE accumulates accT = [Wh|1]^T @ P (16-bit stationary, P moving,
#   1 cyc/row).  Row O of accT is the softmax denominator (ones column).
#   b is folded into stage 3 (h' = num/den + b since att sums to 1).
#   elu(x) = max(x, exp(min(x,0))-1).
#
# kernel() accepts the original full inputs and re-lays them out on host:
# h -> hT (transposed per graph), adj -> int8/bf16.  Pure layout/dtype
# changes; all math runs on device.

import numpy as np
import ml_dtypes

import concourse.bacc as bacc
import concourse.mybir as mybir
import concourse.tile as tile
from concourse import masks
from concourse import dve_ops as dvo
from concourse.dve_spec import (
    Spec, Src0, Src1, Zero, C0, C1, maxx, select,
    _has_src1 as has_src1, lower as dve_lower,
)
from concourse.dve_uop import DveOpSpec
from concourse.bass_utils import run_bass_kernel_spmd
from contextlib import ExitStack


def _register_gat_sep():
    """Custom DVE op: P = select(adj != 0, max(in0*s0, s1), 0)."""
    name = "GAT_SEP_MASK_ANT"
    for o in dvo.OPS:
        if o.name == name:
            return o
    body = select(Src1, maxx(Src0 * C0, C1), Zero)

    def _ref(in0, in1, s0, s1, imm2):
        return np.where(in1 != 0,
                        np.maximum(in0.astype(np.float32) * s0, s1),
                        np.float32(0.0)).astype(np.float32)

    spec = Spec(body=body, reference=_ref)
    row = dvo._CUSTOM_DVE_ROW_BASE + len(dvo.OPS)
    assert row < 0x20, "custom DVE opcode rows exhausted"
    shas = {}
    for ver in ("v3", "v4"):
        tmp = DveOpSpec(name=name, opcode=row, uops=dve_lower(spec, ver=ver),
                        rd1_en=has_src1(spec))
        shas[ver] = tmp.sha(ver)
    op = dvo.DveOp(name, spec, subdim=False, uops_sha=shas)
    dvo.OPS.append(op)
    dvo._SUB_OPCODE_FOR_NAME[name] = row
    return op


GAT_SEP = _register_gat_sep()

N = 2048
F = 128
O = 64
B = 8
ALPHA = 0.2

F32 = mybir.dt.float32
F32R = mybir.dt.float32r
FP16 = mybir.dt.float16
BF16 = mybir.dt.bfloat16
I8 = mybir.dt.int8
AF = mybir.ActivationFunctionType
ALU = mybir.AluOpType
AX = mybir.AxisListType

RT = N // 128   # 16 row blocks of 128
CW = 512        # matmul chunk width (one PSUM bank of fp32)
CT = N // CW    # 4 chunks
ET = N // 128   # 16 epilogue chunks
RB = 2          # row blocks merged per adjacency DMA

WCOL = O + 3    # [W | 0 | w1 | w2]; pw cols: Wh, ones-src, s1, s2


def build_gat_kernel(repeat=1, hw_loop=False, dma_only=False, adj_bufs=3,
                     bf16_path=True, pool_cols=256):
    nc = bacc.Bacc("TRN2", target_bir_lowering=False, debug=False, num_devices=B)

    adj_dt = BF16 if bf16_path else I8
    p_dt = BF16 if bf16_path else F32R
    wh_dt = FP16 if bf16_path else F32R

    hT = nc.dram_tensor("hT", [F, N], F32, kind="ExternalInput").ap()
    adjm = nc.dram_tensor("adjm", [N, N], adj_dt, kind="ExternalInput").ap()
    W = nc.dram_tensor("W", [F, O], F32, kind="ExternalInput").ap()
    bvec = nc.dram_tensor("b", [O], F32, kind="ExternalInput").ap()
    avec = nc.dram_tensor("a", [2 * O, 1], F32, kind="ExternalInput").ap()
    y = nc.dram_tensor("y", [N, O], F32, kind="ExternalOutput").ap()

    XC = pool_cols          # trailing columns of each P block done on Pool
    DC = N - XC

    with tile.TileContext(nc) as tc, ExitStack() as ctx:
        const = ctx.enter_context(tc.tile_pool(name="const", bufs=1))
        ld = ctx.enter_context(tc.tile_pool(name="ld", bufs=2))
        ps = ctx.enter_context(tc.tile_pool(name="ps", bufs=2, space="PSUM"))
        ps_acc = ctx.enter_context(tc.tile_pool(name="ps_acc", bufs=1, space="PSUM"))
        ps_ep = ctx.enter_context(tc.tile_pool(name="ps_ep", bufs=2, space="PSUM"))
        adj_pool = ctx.enter_context(tc.tile_pool(name="adjp", bufs=adj_bufs))
        p_pool = ctx.enter_context(tc.tile_pool(name="pp", bufs=3))
        ep_pool = ctx.enter_context(tc.tile_pool(name="epp", bufs=4))
        it_pool = ctx.enter_context(tc.tile_pool(name="iter", bufs=2))

        # ---------- one-time constants ----------
        ident = const.tile([128, 128], F32)
        masks.make_identity(nc, ident[:])

        ones_row = const.tile([1, 128], F32)
        nc.vector.memset(ones_row[:], 1.0)
        ones_row_r = const.tile([1, 128], F32R)
        nc.vector.tensor_copy(ones_row_r[:], ones_row[:])

        W_sb = const.tile([F, O], F32)
        nc.sync.dma_start(W_sb[:], W)
        a1_sb = const.tile([O, 1], F32)
        nc.sync.dma_start(a1_sb[:], avec[:O, :])
        a2_sb = const.tile([O, 1], F32)
        nc.sync.dma_start(a2_sb[:], avec[O:, :])
        bcol_sb = const.tile([O, 1], F32)
        nc.sync.dma_start(bcol_sb[:], bvec[:, None])

        # W3 = [W | 0 | w1 | w2]: one stage-1 matmul per row block gives
        # [Wh-partial | 0 | s1 | s2].
        W3_sb = const.tile([F, WCOL], F32)
        nc.vector.memset(W3_sb[:, O:O + 1], 0.0)
        nc.sync.dma_start(W3_sb[:, :O], W)
        pwt = ps.tile([O, 128], F32, tag="s1ps")
        nc.tensor.transpose(pwt[:], W_sb[:], ident[:])
        wT = const.tile([O, 128], F32)
        nc.scalar.copy(wT[:], pwt[:])
        pw1 = ps.tile([128, 1], F32, tag="s1ps")
        nc.tensor.matmul(pw1[:], wT[:], a1_sb[:], start=True, stop=True)
        nc.scalar.copy(W3_sb[:, O + 1:O + 2], pw1[:])
        pw2 = ps.tile([128, 1], F32, tag="s1ps")
        nc.tensor.matmul(pw2[:], wT[:], a2_sb[:], start=True, stop=True)
        nc.scalar.copy(W3_sb[:, O + 2:O + 3], pw2[:])

        # ones column for the denominator: [0-col in W3] + 1 via pw add
        one_col = const.tile([128, 1], F32)
        nc.vector.memset(one_col[:], 1.0)

        # b broadcast [128, O] for the stage-3 h' + b fold
        b_row = const.tile([1, O], F32)
        nc.sync.dma_start(b_row[:], bvec[None, :])
        b_bc = const.tile([128, O], F32)
        pbb = ps.tile([128, O], F32, tag="s1ps")
        nc.tensor.matmul(pbb[:], ones_row[:], b_row[:], start=True, stop=True)
        nc.scalar.copy(b_bc[:], pbb[:])

        # beta1 = b.a1 broadcast [128,1] (and 0.2x); beta2 = b.a2 [1,1] (0.8x)
        pb1 = ps.tile([1, 1], F32, tag="s1ps")
        nc.tensor.matmul(pb1[:], bcol_sb[:], a1_sb[:], start=True, stop=True)
        b1_sb = const.tile([1, 1], F32)
        nc.scalar.copy(b1_sb[:], pb1[:])
        pb1b = ps.tile([128, 1], F32, tag="s1ps")
        nc.tensor.matmul(pb1b[:], ones_row[:], b1_sb[:], start=True, stop=True)
        b1_bc = const.tile([128, 1], F32)
        nc.scalar.copy(b1_bc[:], pb1b[:])
        b1f_bc = const.tile([128, 1], F32)
        nc.scalar.mul(b1f_bc[:], pb1b[:], ALPHA)
        pb2 = ps.tile([1, 1], F32, tag="s1ps")
        nc.tensor.matmul(pb2[:], bcol_sb[:], a2_sb[:], start=True, stop=True)
        b2_sb = const.tile([1, 1], F32)
        nc.scalar.copy(b2_sb[:], pb2[:])
        pb2b = ps.tile([128, 1], F32, tag="s1ps")
        nc.tensor.matmul(pb2b[:], ones_row[:], b2_sb[:], start=True, stop=True)
        b2f8_bc = const.tile([128, 1], F32)
        nc.scalar.mul(b2f8_bc[:], pb2b[:], 1.0 - ALPHA)

        # Warm the Exp activation-table set so the ~2.7us table load
        # overlaps the first DMAs instead of stalling the first exp.
        warm = const.tile([1, 1], F32)
        nc.scalar.activation(warm[:], ones_row[:, :1], AF.Exp)

        def _body(_iv=None):
            s12_all = it_pool.tile([128, RT, 2], F32, tag="s12")
            q2_row = it_pool.tile([1, N], F32R, tag="q2row")
            q2_bc = it_pool.tile([128, N], p_dt if bf16_path else F32,
                                 tag="q2bc")
            accT = it_pool.tile([O + 1, N], F32, tag="accT")
            out_sb = it_pool.tile([128, ET, O], F32, tag="outsb")
            hl = ld.tile([128, N], F32, tag="hT")
            HH = N // 2
            nc.sync.dma_start(hl[:, :HH], hT[:, :HH])
            nc.sync.dma_start(hl[:, HH:], hT[:, HH:])

            # ---------- stage 1: Wh, s1, s2 ----------
            whms = []
            rng = range(RT) if not dma_only else range(0)
            for r in rng:
                rsl = slice(r * 128, (r + 1) * 128)
                pw = ps.tile([128, WCOL], F32, tag="s1ps")
                nc.tensor.matmul(pw[:], hl[:, rsl], W3_sb[:], start=True,
                                 stop=True)
                # ones column: add 1 to the zero col during the PSUM read
                whm = it_pool.tile([128, O + 1], wh_dt, tag=f"whm{r}")
                nc.scalar.copy(whm[:, :O], pw[:, :O])
                nc.scalar.activation(whm[:, O:O + 1], pw[:, O:O + 1],
                                     AF.Identity, bias=one_col[:])
                nc.scalar.copy(s12_all[:, r, :], pw[:, O + 1:O + 3])
                whms.append(whm)

            e1s, f1s = [], []
            if not dma_only:
                # q2 = exp(0.8 s2 + 0.8 beta2) on [128,RT] (cheap, 128 lanes),
                # then transpose + tiny SBUF DMA gathers it into a [1,N] row.
                q2a = ep_pool.tile([128, RT], F32, tag="q2a")
                nc.scalar.activation(q2a[:], s12_all[:, :, 1], AF.Exp,
                                     bias=b2f8_bc[:], scale=1.0 - ALPHA)
                ps2 = ps.tile([RT, 128], F32, tag="s1ps")
                nc.tensor.transpose(ps2[:], q2a[:], ident[:])
                st = ep_pool.tile([RT, 128], F32R, tag="st")
                nc.scalar.copy(st[:], ps2[:])
                nc.sync.dma_start(q2_row[:].rearrange("o (r p) -> o r p", r=RT),
                                  st[:])
                for c in range(CT):
                    csl = slice(c * CW, (c + 1) * CW)
                    pbc = ps.tile([128, CW], F32, tag="s1ps")
                    nc.tensor.matmul(pbc[:], ones_row_r[:], q2_row[:, csl],
                                     start=True, stop=True)
                    nc.scalar.copy(q2_bc[:, csl], pbc[:])
                for r in rng:
                    e1 = ep_pool.tile([128, 1], F32, tag=f"e1_{r%4}")
                    nc.scalar.activation(e1[:], s12_all[:, r, 0:1], AF.Exp,
                                         bias=b1_bc[:], scale=1.0)
                    f1 = ep_pool.tile([128, 1], F32, tag=f"f1_{r%4}")
                    nc.scalar.activation(f1[:], s12_all[:, r, 0:1], AF.Exp,
                                         bias=b1f_bc[:], scale=ALPHA)
                    e1s.append(e1)
                    f1s.append(f1)

            # ---------- stage 2: P per block, accT += whm^T @ P ----------
            accs = []
            for c in range(CT) if not dma_only else range(0):
                acc_c = ps_acc.tile([O + 1, CW], F32, tag=f"acc{c}")
                accs.append(acc_c)
            adj_blk = adjm.rearrange("(blk rb p) n -> blk p rb n", rb=RB, p=128)
            for blk in range(RT // RB):
                adj_t = adj_pool.tile([128, RB, N], adj_dt)
                dma_eng = nc.sync if blk % 2 == 0 else nc.scalar
                dma_eng.dma_start(adj_t[:], adj_blk[blk])
                if dma_only:
                    continue
                for rb in range(RB):
                    r = blk * RB + rb
                    p = p_pool.tile([128, N], p_dt, tag="p")
                    if bf16_path:
                        pu = p_pool.tile([128, N], BF16, tag="pu")
                        nc.vector.tensor_scalar(pu[:, :DC], q2_bc[:, :DC],
                                                e1s[r][:], f1s[r][:],
                                                op0=ALU.mult, op1=ALU.max)
                        nc.vector.tensor_tensor(p[:, :DC], pu[:, :DC],
                                                adj_t[:, rb, :DC], op=ALU.mult)
                        if XC:
                            nc.gpsimd.tensor_scalar(pu[:, DC:], q2_bc[:, DC:],
                                                    e1s[r][:], f1s[r][:],
                                                    op0=ALU.mult, op1=ALU.max)
                            nc.gpsimd.tensor_tensor(p[:, DC:], pu[:, DC:],
                                                    adj_t[:, rb, DC:],
                                                    op=ALU.mult)
                    else:
                        nc.vector._custom_dve(
                            GAT_SEP, out=p[:, :DC], in0=q2_bc[:, :DC],
                            in1=adj_t[:, rb, :DC], s0=e1s[r][:], s1=f1s[r][:])
                        if XC:
                            pu = p_pool.tile([128, N], F32, tag="pu")
                            nc.gpsimd.tensor_scalar(pu[:, DC:], q2_bc[:, DC:],
                                                    e1s[r][:], f1s[r][:],
                                                    op0=ALU.mult, op1=ALU.max)
                            nc.gpsimd.tensor_tensor(p[:, DC:], pu[:, DC:],
                                                    adj_t[:, rb, DC:],
                                                    op=ALU.mult)
                    for c in range(CT):
                        csl = slice(c * CW, (c + 1) * CW)
                        nc.tensor.matmul(accs[c][:], whms[r][:], p[:, csl],
                                         start=(r == 0), stop=(r == RT - 1))
            if dma_only:
                nc.vector.memset(out_sb[:], 0.0)
                nc.sync.dma_start(y.rearrange("(j p) o -> p j o", p=128),
                                  out_sb[:])
                return
            for c in range(CT):
                nc.scalar.copy(accT[:, c * CW:(c + 1) * CW], accs[c][:])

            # ---------- stage 3: transpose, normalize (+b), ELU, store ----------
            for j in range(ET):
                jsl = slice(j * 128, (j + 1) * 128)
                pt = ps_ep.tile([128, O + 1], F32)
                nc.tensor.transpose(pt[:], accT[:, jsl], ident[:O + 1, :O + 1])
                rec = ep_pool.tile([128, 1], F32, tag="rec")
                nc.vector.reciprocal(rec[:], pt[:, O:O + 1])
                hp0 = ep_pool.tile([128, O], F32, tag="hp0")
                nc.vector.tensor_scalar_mul(hp0[:], pt[:, :O], rec[:])
                hp = ep_pool.tile([128, O], F32, tag="hp")
                nc.gpsimd.tensor_tensor(hp[:], hp0[:], b_bc[:], op=ALU.add)
                mn = ep_pool.tile([128, O], F32, tag="mn")
                nc.gpsimd.tensor_scalar_min(mn[:], hp[:], 0.0)
                g = ep_pool.tile([128, O], F32, tag="g")
                nc.scalar.activation(g[:], mn[:], AF.Exp)
                nc.vector.scalar_tensor_tensor(out_sb[:, j, :], g[:], -1.0,
                                               hp[:], op0=ALU.add, op1=ALU.max)
            nc.sync.dma_start(y.rearrange("(j p) o -> p j o", p=128),
                              out_sb[:])

        if hw_loop and repeat > 1:
            tc.For_i_unrolled(0, repeat, 1, _body, max_unroll=8)
        else:
            for _it in range(repeat):
                _body()

    nc.compile()
    return nc


_NC_CACHE = None
BF16_PATH = True


def prep_inputs(h, adj, W, b, a):
    """Host-side re-layout: h -> per-graph transpose, adj -> bf16/int8."""
    h = np.ascontiguousarray(h, dtype=np.float32)
    W = np.ascontiguousarray(W, dtype=np.float32)
    b = np.ascontiguousarray(b, dtype=np.float32)
    a = np.ascontiguousarray(a, dtype=np.float32)
    hT = np.ascontiguousarray(np.swapaxes(h, -1, -2))
    if BF16_PATH:
        adjm = (np.asarray(adj) != 0).astype(ml_dtypes.bfloat16)
    else:
        adjm = (np.asarray(adj) != 0).astype(np.int8)
    return hT, adjm, W, b, a


def kernel(h, adj, W, b, a):
    global _NC_CACHE
    hT, adjm, W, b, a = prep_inputs(h, adj, W, b, a)

    if _NC_CACHE is None:
        _NC_CACHE = build_gat_kernel(bf16_path=BF16_PATH)
    nc = _NC_CACHE

    in_maps = [
        {"hT": hT[i], "adjm": adjm[i], "W": W, "b": b, "a": a}
        for i in range(B)
    ]
    res = run_bass_kernel_spmd(nc, in_maps, core_ids=list(range(B)))
    out = np.stack([r["y"] for r in res.results], axis=0)
    return out
